# revision 1
# baseline (speedup 1.0000x reference)
"""Trainium2 kernel for nn_LocalEncoder (BLT-style local encoder).

Key structural insight: every per-token quantity (boundary logit z, rmsnorm
scale, q/k/v projections) depends only on the token ID (vocab=260), so all
dense math runs on the 260-row vocab tables instead of 16384 token rows.
Top-k boundary selection ties (same token id => bit-identical z in the fp32
reference) are broken by index, reproduced exactly on the host.

Pipeline:
  Kernel A (8 cores, DF split 8x384): zv partials = w2_slice @ silu(w1_slice @ embT)
  Host:     zv -> per-row boundary selection (stable by (-z, idx)) -> pos/pid/qtok
  Kernel B (8 cores = 4 seqs x 2 query-halves): one-hot gathers of vocab
            q/k/v tables per token, per-token scores + exp on DVE/ACT,
            block-diagonal softmax via one-hot scatter matmuls, wo proj.
"""

import os
import numpy as np
import ml_dtypes

import concourse.bass as bass
import concourse.bacc as bacc
import concourse.mybir as mybir
from concourse.tile import TileContext
from concourse.alu_op_type import AluOpType
from concourse.bass_utils import run_bass_kernel_spmd
from concourse import masks

F32 = mybir.dt.float32
F32R = mybir.dt.float32r
BF16 = mybir.dt.bfloat16
AFT = mybir.ActivationFunctionType
AX = mybir.AxisListType

B, L, D, V, K, H, HD = 4, 4096, 768, 260, 512, 12, 64
DF = 4 * D
VP = 384          # vocab padded to 3 partition chunks
RMS_EPS = 1e-5
NCORES = 8
FSL = DF // NCORES  # 384 f-rows per core in kernel A

_cache = {}


# --------------------------------------------------------------------------- #
# Kernel A: per-core partial zv over a DF slice (fp32 matmuls for precision)
# --------------------------------------------------------------------------- #
def build_kernel_a():
    nc = bacc.Bacc("TRN2", target_bir_lowering=False, debug=False)
    embT_d = nc.dram_tensor("embT", [D, V], F32, kind="ExternalInput")
    w1T_d = nc.dram_tensor("w1T", [D, FSL], F32, kind="ExternalInput")
    b1_d = nc.dram_tensor("b1s", [FSL], F32, kind="ExternalInput")
    w2_d = nc.dram_tensor("w2s", [FSL], F32, kind="ExternalInput")
    zp_d = nc.dram_tensor("zp", [1, V], F32, kind="ExternalOutput")

    with TileContext(nc) as tc:
        with (
            tc.tile_pool(name="sb", bufs=1) as sb,
            tc.tile_pool(name="ps", bufs=2, space="PSUM") as ps,
        ):
            embT = [sb.tile([128, V], F32, tag=f"embT{d}", name=f"embT{d}") for d in range(6)]
            w1T = [sb.tile([128, FSL], F32, tag=f"w1T{d}", name=f"w1T{d}") for d in range(6)]
            for d in range(6):
                nc.sync.dma_start(embT[d][:, :], embT_d[128 * d:128 * (d + 1), :])
                nc.sync.dma_start(w1T[d][:, :], w1T_d[128 * d:128 * (d + 1), :])
            b1c = sb.tile([128, 3], F32, tag="b1c")
            w2c = sb.tile([128, 3], F32, tag="w2c")
            nc.sync.dma_start(b1c[:, :], b1_d.rearrange("(i p) -> p i", p=128))
            nc.sync.dma_start(w2c[:, :], w2_d.rearrange("(i p) -> p i", p=128))

            zp_ps = ps.tile([1, V], F32, tag="zp")
            for fi in range(3):
                y1p = ps.tile([128, V], F32, tag="y1")
                for d in range(6):
                    nc.tensor.matmul(
                        y1p[:, :], w1T[d][:, 128 * fi:128 * (fi + 1)], embT[d][:, :],
                        start=(d == 0), stop=(d == 5),
                    )
                y1b = sb.tile([128, V], F32, tag="y1b")
                nc.vector.tensor_scalar(y1b[:, :], y1p[:, :], b1c[:, fi:fi + 1],
                                        None, AluOpType.add)
                sig = sb.tile([128, V], F32, tag="sig")
                nc.scalar.activation(sig[:, :], y1b[:, :], AFT.Sigmoid)
                y1s = sb.tile([128, V], F32, tag="y1s")
                nc.vector.tensor_tensor(y1s[:, :], y1b[:, :], sig[:, :],
                                        AluOpType.mult)
                nc.tensor.matmul(zp_ps[:, :], w2c[:, fi:fi + 1], y1s[:, :],
                                 start=(fi == 0), stop=(fi == 2))
            zp_s = sb.tile([1, V], F32, tag="zps")
            nc.vector.tensor_copy(zp_s[:, :], zp_ps[:, :])
            nc.sync.dma_start(zp_d[:, :], zp_s[:, :])
    nc.compile()
    return nc


def run_kernel_a(inputs):
    if "A" not in _cache:
        _cache["A"] = build_kernel_a()
    nc = _cache["A"]
    embT = np.ascontiguousarray(inputs["embed_W"].astype(np.float32).T)
    w1 = inputs["bp_w1"].astype(np.float32)
    b1 = inputs["bp_b1"].astype(np.float32)
    w2 = inputs["bp_w2"].astype(np.float32)[0]
    in_maps = []
    for c in range(NCORES):
        sl = slice(c * FSL, (c + 1) * FSL)
        in_maps.append({
            "embT": embT,
            "w1T": np.ascontiguousarray(w1[sl].T),
            "b1s": np.ascontiguousarray(b1[sl]),
            "w2s": np.ascontiguousarray(w2[sl]),
        })
    res = run_bass_kernel_spmd(nc, in_maps, list(range(NCORES)),
                               trace=os.environ.get("KERNEL_TRACE") == "1")
    _cache["tA"] = res.exec_time_ns
    zv = np.zeros(V, np.float64)
    for c in range(NCORES):
        zv += res.results[c]["zp"][0].astype(np.float64)
    zv += inputs["bp_b2"].astype(np.float64)[0]
    return zv.astype(np.float32)


# --------------------------------------------------------------------------- #
# Host boundary logic
# --------------------------------------------------------------------------- #
def boundary_plan(zv, tokens):
    """Reproduce reference top-k (stable ties by index) + patch structure."""
    zt = zv[tokens]  # [B, L]
    pos = np.zeros((B, K), np.int64)
    for b in range(B):
        key = zt[b].astype(np.float64).copy()
        key[0] = np.inf  # position 0 forced boundary (logprob set to 0 = max)
        order = np.lexsort((np.arange(L), -key))
        pos[b] = np.sort(order[:K])
    pid = (pos[:, None, :] <= np.arange(L)[None, :, None]).sum(-1) - 1  # [B, L]
    qtok = np.take_along_axis(tokens, np.take_along_axis(pos, pid, 1), 1)  # [B, L]
    return pos, pid, qtok


# --------------------------------------------------------------------------- #
# Kernel B: sparse cross-attention via vocab tables + one-hot matmuls
# --------------------------------------------------------------------------- #
def build_kernel_b(jobs):
    """jobs: per-core dict with j0 and per-jc tchunk ranges (python ints ->
    data-dependent instruction stream; same NEFF runs on all 8 cores with the
    max structure, masking handles core differences).  To keep one NEFF for
    all cores, we use the UNION structure: every core runs the same tchunk
    count per jc slot; tchunk indices and j0 are per-core DATA (iota bases
    must be static though) -- so instead we compile per-core variants only if
    structure differs.  Simpler: compile ONE program parameterized by the max
    chunk counts; per-core tchunk starts enter via DRAM-provided pid/tok/qtok
    columns (already per-core rebased by host).
    """
    n0, n1 = jobs["n0"], jobs["n1"]  # tchunks for jc0 / jc1 (uniform, padded)
    nc = bacc.Bacc("TRN2", target_bir_lowering=False, debug=False)

    # vocab tables
    emb_d = nc.dram_tensor("emb", [VP, D], F32, kind="ExternalInput")
    embT_d = nc.dram_tensor("embT", [D, VP], BF16, kind="ExternalInput")
    wqT_d = nc.dram_tensor("wqT", [D, D], BF16, kind="ExternalInput")
    wkT_d = nc.dram_tensor("wkT", [D, D], BF16, kind="ExternalInput")
    wvT_d = nc.dram_tensor("wvT", [D, D], BF16, kind="ExternalInput")
    woT_d = nc.dram_tensor("woT", [D, D], BF16, kind="ExternalInput")
    # per-core token structure, already sliced/padded by host:
    # rows: [1, NT*128] token ids / boundary-token ids (f32), NT = n0+n1
    NT = n0 + n1
    tqr_d = nc.dram_tensor("tqr", [1, NT * 256], F32R, kind="ExternalInput")
    pidc_d = nc.dram_tensor("pidc", [NT * 128], F32, kind="ExternalInput")
    out_d = nc.dram_tensor("out", [256, D], F32, kind="ExternalOutput")

    with TileContext(nc) as tc:
        with (
            tc.tile_pool(name="sb", bufs=1) as sb,
            tc.tile_pool(name="wk", bufs=1) as wkp,
            tc.tile_pool(name="ps", bufs=1, space="PSUM") as ps,
            tc.tile_pool(name="acc", bufs=1, space="PSUM") as accp,
        ):
            # ---- global small tiles ----
            ones_f = sb.tile([1, 128], F32, tag="onesf")
            nc.gpsimd.memset(ones_f[:, :], 1.0)
            ones_r = sb.tile([1, 128], F32R, tag="onesr")
            nc.vector.tensor_copy(ones_r[:, :], ones_f[:, :])
            ident = sb.tile([128, 128], F32, tag="ident")
            masks.make_identity(nc, ident[:, :])
            iotav = sb.tile([128, 3], F32, tag="iotav")  # col p+0/128/256
            for vc in range(3):
                nc.gpsimd.iota(iotav[:, vc:vc + 1], [[0, 1]], base=128 * vc,
                               channel_multiplier=1,
                               allow_small_or_imprecise_dtypes=True)
            iotaj = [sb.tile([128, 128], F32, tag=f"iotaj{jc}", name=f"iotaj{jc}") for jc in range(2)]
            for jc in range(2):
                # value = j0 + jc*128 + f ; j0 enters via host-rebased pid
                nc.gpsimd.iota(iotaj[jc][:, :], [[1, 128]], base=128 * jc,
                               channel_multiplier=0,
                               allow_small_or_imprecise_dtypes=True)

            # ---- load weights / tables ----
            emb = [sb.tile([128, D], F32, tag=f"emb{v}", name=f"emb{v}") for v in range(3)]
            for v in range(3):
                nc.sync.dma_start(emb[v][:, :], emb_d[128 * v:128 * (v + 1), :])
            embT = [sb.tile([128, VP], BF16, tag=f"embT{d}", name=f"embTb{d}") for d in range(6)]
            for d in range(6):
                nc.sync.dma_start(embT[d][:, :], embT_d[128 * d:128 * (d + 1), :])
            wts = {}
            for nm, dd in (("wq", wqT_d), ("wk", wkT_d), ("wv", wvT_d), ("wo", woT_d)):
                wts[nm] = [wkp.tile([128, D], BF16, tag=f"{nm}{d}", name=f"{nm}_{d}") for d in range(6)]
                for d in range(6):
                    nc.sync.dma_start(wts[nm][d][:, :], dd[128 * d:128 * (d + 1), :])

            # per-tchunk structure columns
            pidc = sb.tile([128, NT], F32, tag="pidc")
            nc.sync.dma_start(pidc[:, :], pidc_d.rearrange("(i p) -> p i", p=128))

            # ---- rmsnorm scales: rv (k/v), rv8 (q, includes /8) ----
            msq = sb.tile([128, 3], F32, tag="msq")
            sqjunk = sb.tile([128, D], F32, tag="sqjunk")
            for v in range(3):
                nc.scalar.activation(sqjunk[:, :], emb[v][:, :], AFT.Square,
                                     accum_out=msq[:, v:v + 1])
            # rv = (msq/768 + eps)^-1/2 = exp(-0.5*ln(msq/768 + eps))
            epsc = sb.tile([128, 1], F32, tag="epsc")
            nc.gpsimd.memset(epsc[:, :], RMS_EPS)
            lnv = sb.tile([128, 3], F32, tag="lnv")
            nc.scalar.activation(lnv[:, :], msq[:, :], AFT.Ln,
                                 scale=1.0 / D, bias=epsc[:, :1])
            rv = sb.tile([128, 3], F32, tag="rv")
            nc.scalar.activation(rv[:, :], lnv[:, :], AFT.Exp, scale=-0.5)
            rv8 = sb.tile([128, 3], F32, tag="rv8")
            nc.vector.tensor_scalar(rv8[:, :], rv[:, :], 0.125, None, AluOpType.mult)

            # ---- vocab tables q_s / k_n / v_n [3][128, D] f32r ----
            tabs = {}
            for nm, wname, scl in (("q", "wq", rv8), ("k", "wk", rv), ("v", "wv", rv)):
                tabs[nm] = []
                for v in range(3):
                    tp = ps.tile([128, D], F32, tag="qg", name="tp")
                    for d in range(6):
                        nc.tensor.matmul(
                            tp[:, :512], embT[d][:, 128 * v:128 * (v + 1)],
                            wts[wname][d][:, :512], start=(d == 0), stop=(d == 5))
                        nc.tensor.matmul(
                            tp[:, 512:], embT[d][:, 128 * v:128 * (v + 1)],
                            wts[wname][d][:, 512:], start=(d == 0), stop=(d == 5))
                    ts_ = sb.tile([128, D], BF16, tag=f"tab{nm}{v}")
                    nc.vector.tensor_scalar(ts_[:, :], tp[:, :], scl[:, v:v + 1],
                                            None, AluOpType.mult)
                    tabs[nm].append(ts_)

            # ---- main loop: two query chunks ----
            for jc in range(2):
                ntc = n0 if jc == 0 else n1
                base = 0 if jc == 0 else n0
                acc = accp.tile([128, 1536], F32, tag="acc", name="acc")
                for i in range(ntc):
                    tci = base + i
                    # broadcast token+qtok rows across partitions (one matmul)
                    tq_s = sb.tile([1, 256], F32R, tag="tokslice", name="tq_s", bufs=3)
                    nc.sync.dma_start(tq_s[:, :], tqr_d[:, 256 * tci:256 * (tci + 1)])
                    btok2 = ps.tile([128, 256], F32, tag="btok", name="btok2")
                    nc.tensor.matmul(btok2[:, :], ones_r[:, :], tq_s[:, :],
                                     start=True, stop=True)
                    btok = btok2[:, :128]
                    bqtok = btok2[:, 128:]
                    ohk = []
                    ohq = []
                    for v in range(3):
                        o1 = sb.tile([128, 128], BF16, tag=f"ohk{v}", name=f"o1_{v}", bufs=2)
                        nc.vector.tensor_scalar(o1[:, :], btok,
                                                iotav[:, v:v + 1], None,
                                                AluOpType.is_equal)
                        ohk.append(o1)
                        o2 = sb.tile([128, 128], BF16, tag=f"ohq{v}", name=f"o2_{v}", bufs=2)
                        nc.vector.tensor_scalar(o2[:, :], bqtok,
                                                iotav[:, v:v + 1], None,
                                                AluOpType.is_equal)
                        ohq.append(o2)
                    # gathers: qg/kg/vg [t,768]
                    qg = ps.tile([128, D], F32, tag="qg")
                    kg = ps.tile([128, D], F32, tag="kg")
                    for v in range(3):
                        nc.tensor.matmul(qg[:, :512], ohq[v][:, :],
                                         tabs["q"][v][:, :512],
                                         start=(v == 0), stop=(v == 2))
                        nc.tensor.matmul(qg[:, 512:], ohq[v][:, :],
                                         tabs["q"][v][:, 512:],
                                         start=(v == 0), stop=(v == 2))
                        nc.tensor.matmul(kg[:, :512], ohk[v][:, :],
                                         tabs["k"][v][:, :512],
                                         start=(v == 0), stop=(v == 2))
                        nc.tensor.matmul(kg[:, 512:], ohk[v][:, :],
                                         tabs["k"][v][:, 512:],
                                         start=(v == 0), stop=(v == 2))
                    # scores + exp
                    kgs = sb.tile([128, D], F32, tag="kgs", bufs=2)
                    nc.scalar.copy(kgs[:, :], kg[:, :])
                    prod = sb.tile([128, D], F32, tag="prod", bufs=2)
                    nc.vector.tensor_tensor(prod[:, :], qg[:, :], kgs[:, :],
                                            AluOpType.mult)
                    s12 = sb.tile([128, H], F32, tag="s12", bufs=2)
                    nc.vector.tensor_reduce(
                        ap3(s12, H, 1), ap3(prod, H, HD), AX.X, AluOpType.add)
                    e12f = sb.tile([128, H], F32, tag="e12f", bufs=2)
                    nc.scalar.activation(e12f[:, :], s12[:, :], AFT.Exp)
                    e12 = sb.tile([128, H], BF16, tag="e12", bufs=2)
                    nc.vector.tensor_copy(e12[:, :], e12f[:, :])
                    # value gather (reuses qg slot) and weight
                    vg = ps.tile([128, D], F32, tag="qg")
                    for v in range(3):
                        nc.tensor.matmul(vg[:, :512], ohk[v][:, :],
                                         tabs["v"][v][:, :512],
                                         start=(v == 0), stop=(v == 2))
                        nc.tensor.matmul(vg[:, 512:], ohk[v][:, :],
                                         tabs["v"][v][:, 512:],
                                         start=(v == 0), stop=(v == 2))
                    wv = sb.tile([128, D], BF16, tag="wv", bufs=2)
                    nc.vector.tensor_tensor(ap3(wv, H, HD),
                                            bcast3(e12f, H, HD),
                                            ap3(vg, H, HD), AluOpType.mult)
                    # membership MT [t, j] and scatter
                    mt = sb.tile([128, 128], BF16, tag="mt", bufs=2)
                    nc.vector.tensor_scalar(mt[:, :], iotaj[jc][:, :],
                                            pidc[:, tci:tci + 1], None,
                                            AluOpType.is_equal)
                    nc.tensor.matmul(acc[:, :512], mt[:, :], wv[:, :512],
                                     start=(i == 0), stop=(i == ntc - 1))
                    nc.tensor.matmul(acc[:, 512:768], mt[:, :], wv[:, 512:],
                                     start=(i == 0), stop=(i == ntc - 1))
                    nc.tensor.matmul(acc[:, 1024:1036], mt[:, :], e12[:, :],
                                     start=(i == 0), stop=(i == ntc - 1))
                # ---- finalize jc ----
                lnz = sb.tile([128, H], F32, tag="lnz")
                nc.scalar.activation(lnz[:, :], acc[:, 1024:1036], AFT.Ln)
                zrec = sb.tile([128, H], F32, tag="zrec")
                nc.scalar.activation(zrec[:, :], lnz[:, :], AFT.Exp, scale=-1.0)
                pr = sb.tile([128, D], F32, tag="pr")
                nc.vector.tensor_tensor(ap3(pr, H, HD), bcast3(zrec, H, HD),
                                        ap3(acc, H, HD, width=780), AluOpType.mult)
                fin = ps.tile([128, D], F32, tag="kg", name="fin")
                for d in range(6):
                    trp = ps.tile([128, 128], F32, tag="btok")
                    nc.tensor.transpose(trp[:, :], pr[:, 128 * d:128 * (d + 1)],
                                        ident[:, :])
                    trs = sb.tile([128, 128], BF16, tag="trs", bufs=2)
                    nc.vector.tensor_copy(trs[:, :], trp[:, :])
                    nc.tensor.matmul(fin[:, :512], trs[:, :], wts["wo"][d][:, :512],
                                     start=(d == 0), stop=(d == 5))
                    nc.tensor.matmul(fin[:, 512:], trs[:, :], wts["wo"][d][:, 512:],
                                     start=(d == 0), stop=(d == 5))
                fin_s = sb.tile([128, D], F32, tag="fins")
                nc.vector.tensor_copy(fin_s[:, :], fin[:, :])
                nc.sync.dma_start(out_d[128 * jc:128 * (jc + 1), :], fin_s[:, :])
    nc.compile()
    return nc


def ap3(tile, n, w, width=None):
    """[128, n*w] tile viewed as [128, n, w] (first n*w cols)."""
    p = tile.ap[0] if hasattr(tile, "ap") else None
    t = tile[:, :]
    ps, fs = t.ap[0], t.ap[1]
    return bass.AP(t.tensor, t.offset, [list(ps), [fs[0] * w, n], [fs[0], w]])


def bcast3(tile, n, w):
    """[128, n] tile broadcast to [128, n, w] via 0-stride inner dim."""
    t = tile[:, :]
    ps, fs = t.ap[0], t.ap[1]
    return bass.AP(t.tensor, t.offset, [list(ps), [fs[0], n], [0, w]])


# --------------------------------------------------------------------------- #
# top-level
# --------------------------------------------------------------------------- #
def kernel(tokens, embed_W, bp_w1, bp_b1, bp_w2, bp_b2, wq, wk, wv, wo,
           qnorm_w, kvnorm_w, k_patches):
    tokens = np.asarray(tokens).astype(np.int64)
    inputs = dict(tokens=tokens, embed_W=embed_W, bp_w1=bp_w1, bp_b1=bp_b1,
                  bp_w2=bp_w2, bp_b2=bp_b2)
    zv = run_kernel_a(inputs)
    pos, pid, qtok = boundary_plan(zv, tokens)

    # per-core job structure: core = 2*b + half; queries [half*256, half*256+256)
    cores = []
    for b in range(B):
        for half in range(2):
            j0 = half * 256
            ends = [pos[b, j0 + 128] if j0 + 128 < K else L,
                    pos[b, j0 + 256] if j0 + 256 < K else L]
            starts = [pos[b, j0], pos[b, j0 + 128] if j0 + 128 < K else L]
            tcs = []
            for jc in range(2):
                lo, hi = int(starts[jc]) // 128, -(-int(ends[jc]) // 128)
                tcs.append(list(range(lo, max(hi, lo + 1))))
            cores.append({"b": b, "j0": j0, "tcs": tcs})
    n0 = max(len(c["tcs"][0]) for c in cores)
    n1 = max(len(c["tcs"][1]) for c in cores)
    key = ("B", n0, n1)
    if key not in _cache:
        _cache[key] = build_kernel_b({"n0": n0, "n1": n1})
    nc = _cache[key]

    # host-side weight prep (norm-weight folding only)
    embp = np.zeros((VP, D), np.float32)
    embp[:V] = embed_W.astype(np.float32)
    embTp = np.ascontiguousarray(embp.T).astype(ml_dtypes.bfloat16)
    wq_f = np.ascontiguousarray((wq.astype(np.float32)
                                 * qnorm_w.astype(np.float32)[None, :]).T).astype(ml_dtypes.bfloat16)
    wk_f = np.ascontiguousarray((wk.astype(np.float32)
                                 * kvnorm_w.astype(np.float32)[None, :]).T).astype(ml_dtypes.bfloat16)
    wv_f = np.ascontiguousarray((wv.astype(np.float32)
                                 * kvnorm_w.astype(np.float32)[None, :]).T).astype(ml_dtypes.bfloat16)
    wo_f = np.ascontiguousarray(wo.astype(np.float32).T).astype(ml_dtypes.bfloat16)

    NT = n0 + n1
    in_maps = []
    for c in cores:
        b = c["b"]
        tqr = np.zeros(NT * 256, np.float32)
        pidc = np.full(NT * 128, -1.0, np.float32)  # -1 never matches a j id
        slot = 0
        for jc in range(2):
            lst = c["tcs"][jc]
            # pad each jc segment to its uniform length with repeats of the
            # first chunk (harmless: pid mask kills contributions, and for
            # padded slots we also set pid=-1)
            want = n0 if jc == 0 else n1
            for k_ in range(want):
                if k_ < len(lst):
                    tci = lst[k_]
                    sl = slice(tci * 128, (tci + 1) * 128)
                    tqr[slot * 256:slot * 256 + 128] = tokens[b, sl]
                    tqr[slot * 256 + 128:(slot + 1) * 256] = qtok[b, sl]
                    # rebase pid to local j index (0..255 within this core)
                    pidc[slot * 128:(slot + 1) * 128] = pid[b, sl] - c["j0"]
                slot += 1
        in_maps.append({
            "emb": embp, "embT": embTp, "wqT": wq_f, "wkT": wk_f,
            "wvT": wv_f, "woT": wo_f,
            "tqr": tqr[None, :], "pidc": pidc,
        })
    res = run_bass_kernel_spmd(nc, in_maps, list(range(NCORES)),
                               trace=os.environ.get("KERNEL_TRACE") == "1")
    _cache["tB"] = res.exec_time_ns
    out = np.zeros((B, K, D), np.float32)
    for ci, c in enumerate(cores):
        out[c["b"], c["j0"]:c["j0"] + 256] = res.results[ci]["out"]
    return out



# revision 6
# speedup vs baseline: 1.0066x; 1.0066x over previous
"""Trainium2 kernel for nn_LocalEncoder (BLT-style local encoder).

Key structural insight: every per-token quantity (boundary logit z, rmsnorm
scale, q/k/v projections) depends only on the token ID (vocab=260), so all
dense math runs on the 260-row vocab tables instead of 16384 token rows.
Top-k boundary selection ties (same token id => bit-identical z in the fp32
reference) are broken by index, reproduced exactly on the host.

Pipeline:
  Kernel A (8 cores, DF split 8x384): zv partials = w2_slice @ silu(w1_slice @ embT)
  Host:     zv -> per-row boundary selection (stable by (-z, idx)) -> pos/pid/qtok
  Kernel B (8 cores = 4 seqs x 2 query-halves):
    - rmsnorm scales folded into embT on host; 1/8 and qnorm folded into wq;
      kvnorm folded into wk/wv.
    - preamble: vocab k|v table (fused 1536-wide) + vocab q table -> per-jc
      128-row patch query table qj.
    - per 128-token chunk: one-hot gathers (k|v from vocab, q from qj via the
      patch-membership mask transpose), per-token scores on GpSimd/DVE/ACT,
      block-diagonal softmax via one-hot scatter matmuls, wo proj at end.
    - software-pipelined so PE gathers of chunk i+1 overlap the score chain
      of chunk i (acc matmuls are deferred by one iteration).
"""

import os
import numpy as np
import ml_dtypes

import concourse.bass as bass
import concourse.bacc as bacc
import concourse.mybir as mybir
from concourse.tile import TileContext
from concourse.alu_op_type import AluOpType
from concourse.bass_utils import run_bass_kernel_spmd
from concourse import masks

F32 = mybir.dt.float32
F32R = mybir.dt.float32r
BF16 = mybir.dt.bfloat16
AFT = mybir.ActivationFunctionType
AX = mybir.AxisListType

B, L, D, V, K, H, HD = 4, 4096, 768, 260, 512, 12, 64
DF = 4 * D
VP = 384          # vocab padded to 3 partition chunks
RMS_EPS = 1e-5
NCORES = 8
FSL = DF // NCORES  # 384 f-rows per core in kernel A

_cache = {}


# --------------------------------------------------------------------------- #
# Kernel A: per-core partial zv over a DF slice (fp32 matmuls for precision;
# the top-k threshold gap goes down to ~4e-6 so bf16/f32r would flip
# boundary selections)
# --------------------------------------------------------------------------- #
def build_kernel_a():
    nc = bacc.Bacc("TRN2", target_bir_lowering=False, debug=False)
    embT_d = nc.dram_tensor("embT", [D, V], F32, kind="ExternalInput")
    w1T_d = nc.dram_tensor("w1T", [D, FSL], F32, kind="ExternalInput")
    b1_d = nc.dram_tensor("b1s", [FSL], F32, kind="ExternalInput")
    w2_d = nc.dram_tensor("w2s", [FSL], F32, kind="ExternalInput")
    zp_d = nc.dram_tensor("zp", [1, V], F32, kind="ExternalOutput")

    with TileContext(nc) as tc:
        with (
            tc.tile_pool(name="sb", bufs=1) as sb,
            tc.tile_pool(name="ps", bufs=2, space="PSUM") as ps,
        ):
            embT = [sb.tile([128, V], F32, tag=f"embT{d}", name=f"embT{d}") for d in range(6)]
            w1T = [sb.tile([128, FSL], F32, tag=f"w1T{d}", name=f"w1T{d}") for d in range(6)]
            for d in range(6):
                nc.sync.dma_start(embT[d][:, :], embT_d[128 * d:128 * (d + 1), :])
                nc.sync.dma_start(w1T[d][:, :], w1T_d[128 * d:128 * (d + 1), :])
            b1c = sb.tile([128, 3], F32, tag="b1c")
            w2c = sb.tile([128, 3], F32, tag="w2c")
            nc.sync.dma_start(b1c[:, :], b1_d.rearrange("(i p) -> p i", p=128))
            nc.sync.dma_start(w2c[:, :], w2_d.rearrange("(i p) -> p i", p=128))

            zp_ps = ps.tile([1, V], F32, tag="zp")
            for fi in range(3):
                y1p = ps.tile([128, V], F32, tag="y1")
                for d in range(6):
                    nc.tensor.matmul(
                        y1p[:, :], w1T[d][:, 128 * fi:128 * (fi + 1)], embT[d][:, :],
                        start=(d == 0), stop=(d == 5),
                    )
                y1b = sb.tile([128, V], F32, tag="y1b")
                nc.vector.tensor_scalar(y1b[:, :], y1p[:, :], b1c[:, fi:fi + 1],
                                        None, AluOpType.add)
                sig = sb.tile([128, V], F32, tag="sig")
                nc.scalar.activation(sig[:, :], y1b[:, :], AFT.Sigmoid)
                y1s = sb.tile([128, V], F32, tag="y1s")
                nc.vector.tensor_tensor(y1s[:, :], y1b[:, :], sig[:, :],
                                        AluOpType.mult)
                nc.tensor.matmul(zp_ps[:, :], w2c[:, fi:fi + 1], y1s[:, :],
                                 start=(fi == 0), stop=(fi == 2))
            zp_s = sb.tile([1, V], F32, tag="zps")
            nc.vector.tensor_copy(zp_s[:, :], zp_ps[:, :])
            nc.sync.dma_start(zp_d[:, :], zp_s[:, :])
    nc.compile()
    return nc


def run_kernel_a(inputs):
    if "A" not in _cache:
        _cache["A"] = build_kernel_a()
    nc = _cache["A"]
    embT = np.ascontiguousarray(inputs["embed_W"].astype(np.float32).T)
    w1 = inputs["bp_w1"].astype(np.float32)
    b1 = inputs["bp_b1"].astype(np.float32)
    w2 = inputs["bp_w2"].astype(np.float32)[0]
    in_maps = []
    for c in range(NCORES):
        sl = slice(c * FSL, (c + 1) * FSL)
        in_maps.append({
            "embT": embT,
            "w1T": np.ascontiguousarray(w1[sl].T),
            "b1s": np.ascontiguousarray(b1[sl]),
            "w2s": np.ascontiguousarray(w2[sl]),
        })
    res = run_bass_kernel_spmd(nc, in_maps, list(range(NCORES)),
                               trace=os.environ.get("KERNEL_TRACE") == "1")
    _cache["tA"] = res.exec_time_ns
    zv = np.zeros(V, np.float64)
    for c in range(NCORES):
        zv += res.results[c]["zp"][0].astype(np.float64)
    zv += inputs["bp_b2"].astype(np.float64)[0]
    return zv.astype(np.float32)


# --------------------------------------------------------------------------- #
# Host boundary logic
# --------------------------------------------------------------------------- #
def boundary_plan(zv, tokens):
    """Reproduce reference top-k (stable ties by index) + patch structure."""
    zt = zv[tokens]  # [B, L]
    pos = np.zeros((B, K), np.int64)
    for b in range(B):
        key = zt[b].astype(np.float64).copy()
        key[0] = np.inf  # position 0 forced boundary (logprob set to 0 = max)
        order = np.lexsort((np.arange(L), -key))
        pos[b] = np.sort(order[:K])
    pid = (pos[:, None, :] <= np.arange(L)[None, :, None]).sum(-1) - 1  # [B, L]
    return pos, pid


# --------------------------------------------------------------------------- #
# Kernel B: sparse cross-attention via vocab tables + one-hot matmuls
# --------------------------------------------------------------------------- #
def build_kernel_b(n0, n1):
    """One NEFF for all 8 cores; n0/n1 = uniform (max-padded) chunk counts for
    the two 128-query groups. Per-core data enters via tqr/pidc/qtj."""
    NT = n0 + n1
    nc = bacc.Bacc("TRN2", target_bir_lowering=False, debug=False)

    embT_d = nc.dram_tensor("embT", [D, VP], BF16, kind="ExternalInput")
    wqT_d = nc.dram_tensor("wqT", [D, D], BF16, kind="ExternalInput")
    wkvT_d = nc.dram_tensor("wkvT", [D, 2 * D], BF16, kind="ExternalInput")
    woT_d = nc.dram_tensor("woT", [D, D], BF16, kind="ExternalInput")
    # per chunk slot: [tok(128) | pid_local(128)] as f32
    tqr_d = nc.dram_tensor("tqr", [1, NT * 256], F32R, kind="ExternalInput")
    # boundary-token vocab ids for the 256 patches (jc0 128 | jc1 128)
    qtj_d = nc.dram_tensor("qtj", [1, 256], F32R, kind="ExternalInput")
    pidc_d = nc.dram_tensor("pidc", [NT * 128], F32, kind="ExternalInput")
    out_d = nc.dram_tensor("out", [256, D], F32, kind="ExternalOutput")

    with TileContext(nc) as tc:
        with (
            tc.tile_pool(name="sb", bufs=1) as sb,
            tc.tile_pool(name="wk", bufs=1) as wkp,
            tc.tile_pool(name="ps", bufs=1, space="PSUM") as ps,
        ):
            # ---- constants ----
            ones_f = sb.tile([1, 128], F32, tag="onesf")
            nc.gpsimd.memset(ones_f[:, :], 1.0)
            ones_r = sb.tile([1, 128], F32R, tag="onesr")
            nc.vector.tensor_copy(ones_r[:, :], ones_f[:, :])
            ident = sb.tile([128, 128], F32, tag="ident")
            masks.make_identity(nc, ident[:, :])
            iotav = sb.tile([128, 3], F32, tag="iotav")   # col: p, p+128, p+256
            for vc in range(3):
                nc.gpsimd.iota(iotav[:, vc:vc + 1], [[0, 1]], base=128 * vc,
                               channel_multiplier=1,
                               allow_small_or_imprecise_dtypes=True)
            iotajc = sb.tile([128, 2], F32, tag="iotajc")  # col jc: p + 128*jc
            for jc in range(2):
                nc.gpsimd.iota(iotajc[:, jc:jc + 1], [[0, 1]], base=128 * jc,
                               channel_multiplier=1,
                               allow_small_or_imprecise_dtypes=True)
            iotaj = sb.tile([128, 256], F32, tag="iotaj")  # row 0..255 all parts
            nc.gpsimd.iota(iotaj[:, :], [[1, 256]], base=0,
                           channel_multiplier=0,
                           allow_small_or_imprecise_dtypes=True)

            # ---- load weights / tables (interleaved so build starts early) --
            embT = [sb.tile([128, VP], BF16, tag=f"embT{d}", name=f"embTb{d}") for d in range(6)]
            wq = [wkp.tile([128, D], BF16, tag=f"wq{d}", name=f"wq_{d}") for d in range(6)]
            wkv = [wkp.tile([128, 2 * D], BF16, tag=f"wkv{d}", name=f"wkv_{d}") for d in range(6)]
            wo = [wkp.tile([128, D], BF16, tag=f"wo{d}", name=f"wo_{d}") for d in range(6)]
            for d in range(6):
                nc.sync.dma_start(embT[d][:, :], embT_d[128 * d:128 * (d + 1), :])
                nc.sync.dma_start(wq[d][:, :], wqT_d[128 * d:128 * (d + 1), :])
            for d in range(6):
                nc.sync.dma_start(wkv[d][:, :], wkvT_d[128 * d:128 * (d + 1), :])
            for d in range(6):
                nc.sync.dma_start(wo[d][:, :], woT_d[128 * d:128 * (d + 1), :])
            qtj_s = sb.tile([1, 256], F32R, tag="qtj")
            nc.sync.dma_start(qtj_s[:, :], qtj_d[:, :])
            pidc = sb.tile([128, NT], F32, tag="pidc")
            nc.sync.dma_start(pidc[:, :], pidc_d.rearrange("(i p) -> p i", p=128))

            # ---- vocab q table (only used to extract qj below) ----
            tabq = []
            for v in range(3):
                tp = ps.tile([128, D], F32, tag="qg", name=f"tabq_ps{v}")
                for d in range(6):
                    nc.tensor.matmul(tp[:, :512],
                                     embT[d][:, 128 * v:128 * (v + 1)],
                                     wq[d][:, :512], start=(d == 0), stop=(d == 5))
                    nc.tensor.matmul(tp[:, 512:],
                                     embT[d][:, 128 * v:128 * (v + 1)],
                                     wq[d][:, 512:], start=(d == 0), stop=(d == 5))
                ts_ = sb.tile([128, D], BF16, tag=f"tabq{v}")
                nc.vector.tensor_copy(ts_[:, :], tp[:, :])
                tabq.append(ts_)

            # ---- per-jc 128-row patch query tables qj ----
            btokq = ps.tile([128, 256], F32, tag="btok", name="btokq")
            nc.tensor.matmul(btokq[:, :], ones_r[:, :], qtj_s[:, :],
                             start=True, stop=True)
            qj = []
            for jc in range(2):
                ohqj = []
                for v in range(3):
                    o = sb.tile([128, 128], BF16, tag=f"ohqj{v}_{jc}")
                    nc.vector.tensor_scalar(o[:, :],
                                            btokq[:, 128 * jc:128 * (jc + 1)],
                                            iotav[:, v:v + 1], None,
                                            AluOpType.is_equal)
                    ohqj.append(o)
                qp = ps.tile([128, D], F32, tag="qg", name=f"qj_ps{jc}")
                for v in range(3):
                    nc.tensor.matmul(qp[:, :512], ohqj[v][:, :],
                                     tabq[v][:, :512], start=(v == 0), stop=(v == 2))
                    nc.tensor.matmul(qp[:, 512:], ohqj[v][:, :],
                                     tabq[v][:, 512:], start=(v == 0), stop=(v == 2))
                qs = sb.tile([128, D], BF16, tag=f"qj{jc}")
                nc.vector.tensor_copy(qs[:, :], qp[:, :])
                qj.append(qs)

            # ---- fused vocab k|v table [128, 1536] per vchunk ----
            tabkv = []
            for v in range(3):
                tp = ps.tile([128, 2 * D], F32, tag="kvg", name=f"tabkv_ps{v}")
                for d in range(6):
                    for s in range(3):
                        nc.tensor.matmul(tp[:, 512 * s:512 * (s + 1)],
                                         embT[d][:, 128 * v:128 * (v + 1)],
                                         wkv[d][:, 512 * s:512 * (s + 1)],
                                         start=(d == 0), stop=(d == 5))
                ts_ = sb.tile([128, 2 * D], BF16, tag=f"tabkv{v}")
                nc.vector.tensor_copy(ts_[:, :], tp[:, :])
                tabkv.append(ts_)

            # ---- main loop ----
            def emit_prep(tci, jc):
                """DMA + broadcast + one-hot/mask build for chunk tci."""
                tq_s = sb.tile([1, 256], F32R, tag="tqs", name=f"tqs{tci}", bufs=3)
                nc.sync.dma_start(tq_s[:, :], tqr_d[:, 256 * tci:256 * (tci + 1)])
                bt = ps.tile([128, 256], F32, tag="btok", name=f"btok{tci}")
                nc.tensor.matmul(bt[:, :], ones_r[:, :], tq_s[:, :],
                                 start=True, stop=True)
                ohk = []
                for v in range(3):
                    o = sb.tile([128, 128], BF16, tag=f"ohk{v}", name=f"ohk{v}_{tci}", bufs=2)
                    nc.vector.tensor_scalar(o[:, :], bt[:, :128],
                                            iotav[:, v:v + 1], None,
                                            AluOpType.is_equal)
                    ohk.append(o)
                mtT = sb.tile([128, 128], BF16, tag="mtT", name=f"mtT{tci}", bufs=2)
                nc.vector.tensor_scalar(mtT[:, :], bt[:, 128:],
                                        iotajc[:, jc:jc + 1], None,
                                        AluOpType.is_equal)
                mt = sb.tile([128, 128], BF16, tag="mt", name=f"mt{tci}", bufs=3)
                nc.vector.tensor_scalar(mt[:, :],
                                        iotaj[:, 128 * jc:128 * (jc + 1)],
                                        pidc[:, tci:tci + 1], None,
                                        AluOpType.is_equal)
                return ohk, mtT, mt

            def emit_gather(prep, tci):
                """PE gathers for chunk tci -> (qg, kvg) PSUM."""
                ohk, mtT, mt = prep
                jc = 0 if tci < n0 else 1
                qg = ps.tile([128, D], F32, tag="qg", name=f"qg{tci}")
                nc.tensor.matmul(qg[:, :512], mtT[:, :], qj[jc][:, :512],
                                 start=True, stop=True)
                nc.tensor.matmul(qg[:, 512:], mtT[:, :], qj[jc][:, 512:],
                                 start=True, stop=True)
                kvg = ps.tile([128, 2 * D], F32, tag="kvg", name=f"kvg{tci}")
                for v in range(3):
                    for s in range(3):
                        nc.tensor.matmul(kvg[:, 512 * s:512 * (s + 1)],
                                         ohk[v][:, :],
                                         tabkv[v][:, 512 * s:512 * (s + 1)],
                                         start=(v == 0), stop=(v == 2))
                return qg, kvg

            def emit_score(qg, kvg, tci):
                """score chain -> weighted values wv [128, 780] bf16.

                qg/kvg hold exact bf16 table rows, so the ACT copies to bf16
                SBUF are lossless and release the PSUM banks early (the next
                chunk's gather matmuls reuse them)."""
                qgs = sb.tile([128, D], BF16, tag="qgs", name=f"qgs{tci}", bufs=2)
                nc.scalar.copy(qgs[:, :], qg[:, :])
                kvs = sb.tile([128, 2 * D], BF16, tag="kvs", name=f"kvs{tci}", bufs=2)
                nc.scalar.copy(kvs[:, :], kvg[:, :])
                prod = sb.tile([128, D], F32, tag="prod", name=f"prod{tci}", bufs=2)
                nc.gpsimd.tensor_tensor(prod[:, :], qgs[:, :], kvs[:, :D],
                                        AluOpType.mult)
                s12 = sb.tile([128, H], F32, tag="s12", name=f"s12{tci}", bufs=2)
                nc.vector.tensor_reduce(
                    ap3(s12, H, 1), ap3(prod, H, HD), AX.X, AluOpType.add)
                wv = sb.tile([128, D + H], BF16, tag="wv", name=f"wv{tci}", bufs=2)
                nc.scalar.activation(wv[:, D:], s12[:, :], AFT.Exp)
                e12b = sb.tile([128, H], BF16, tag="e12b", name=f"e12b{tci}", bufs=2)
                nc.scalar.activation(e12b[:, :], s12[:, :], AFT.Exp)
                nc.vector.tensor_tensor(ap3(wv, H, HD),
                                        bcast3(e12b, H, HD),
                                        ap3(kvs, H, HD, off=D), AluOpType.mult)
                return wv

            def emit_acc(acc, mt, wv, first, last):
                nc.tensor.matmul(acc[:, :512], mt[:, :], wv[:, :512],
                                 start=first, stop=last)
                nc.tensor.matmul(acc[:, 512:], mt[:, :], wv[:, 512:],
                                 start=first, stop=last)

            def emit_finalize(acc, jc):
                zrec = sb.tile([128, H], F32, tag="zrec", name=f"zrec{jc}")
                nc.vector.reciprocal(zrec[:, :], acc[:, D:])
                pr = sb.tile([128, D], F32, tag="pr", name=f"pr{jc}")
                nc.vector.tensor_tensor(ap3(pr, H, HD), bcast3(zrec, H, HD),
                                        ap3(acc, H, HD), AluOpType.mult)
                fin = ps.tile([128, D], F32, tag="qg", name=f"fin{jc}")
                for d in range(6):
                    trp = ps.tile([128, 128], F32, tag="btok", name=f"trp{jc}_{d}")
                    nc.tensor.transpose(trp[:, :], pr[:, 128 * d:128 * (d + 1)],
                                        ident[:, :])
                    trs = sb.tile([128, 128], BF16, tag="trs", name=f"trs{jc}_{d}", bufs=2)
                    nc.vector.tensor_copy(trs[:, :], trp[:, :])
                    nc.tensor.matmul(fin[:, :512], trs[:, :], wo[d][:, :512],
                                     start=(d == 0), stop=(d == 5))
                    nc.tensor.matmul(fin[:, 512:], trs[:, :], wo[d][:, 512:],
                                     start=(d == 0), stop=(d == 5))
                fin_s = sb.tile([128, D], F32, tag="fins", name=f"fins{jc}")
                nc.vector.tensor_copy(fin_s[:, :], fin[:, :])
                nc.sync.dma_start(out_d[128 * jc:128 * (jc + 1), :], fin_s[:, :])

            # software-pipelined chunk loop (acc deferred one iteration so the
            # PE never waits on the score chain of the current chunk)
            prep = emit_prep(0, 0)
            for jc in range(2):
                ntc = n0 if jc == 0 else n1
                base = 0 if jc == 0 else n0
                acc = ps.tile([128, D + H], F32, tag="acc", name=f"acc{jc}")
                pending = None  # (mt, wv) awaiting acc
                for i in range(ntc):
                    tci = base + i
                    qg, kvg = emit_gather(prep, tci)
                    mt_i = prep[2]
                    if tci + 1 < NT:
                        prep = emit_prep(tci + 1, 0 if tci + 1 < n0 else 1)
                    if pending is not None:
                        emit_acc(acc, pending[0], pending[1], i == 1, False)
                    wv = emit_score(qg, kvg, tci)
                    pending = (mt_i, wv)
                emit_acc(acc, pending[0], pending[1], ntc == 1, True)
                emit_finalize(acc, jc)
    nc.compile()
    return nc


def ap3(tile, n, w, off=0):
    """[128, >=off+n*w] tile viewed as [128, n, w] starting at col off."""
    t = tile[:, :]
    ps_, fs = t.ap[0], t.ap[1]
    return bass.AP(t.tensor, t.offset + off * fs[0],
                   [list(ps_), [fs[0] * w, n], [fs[0], w]])


def bcast3(tile, n, w, off=0):
    """[128, n] cols (starting at off) broadcast to [128, n, w] via 0-stride."""
    t = tile[:, :]
    ps_, fs = t.ap[0], t.ap[1]
    return bass.AP(t.tensor, t.offset + off * fs[0],
                   [list(ps_), [fs[0], n], [0, w]])


# --------------------------------------------------------------------------- #
# top-level
# --------------------------------------------------------------------------- #
def kernel(tokens, embed_W, bp_w1, bp_b1, bp_w2, bp_b2, wq, wk, wv, wo,
           qnorm_w, kvnorm_w, k_patches):
    tokens = np.asarray(tokens).astype(np.int64)
    inputs = dict(tokens=tokens, embed_W=embed_W, bp_w1=bp_w1, bp_b1=bp_b1,
                  bp_w2=bp_w2, bp_b2=bp_b2)
    zv = run_kernel_a(inputs)
    pos, pid = boundary_plan(zv, tokens)

    # per-core job structure: core = 2*b + half; queries [half*256, half*256+256)
    cores = []
    for b in range(B):
        for half in range(2):
            j0 = half * 256
            ends = [pos[b, j0 + 128] if j0 + 128 < K else L,
                    pos[b, j0 + 256] if j0 + 256 < K else L]
            starts = [pos[b, j0], pos[b, j0 + 128] if j0 + 128 < K else L]
            tcs = []
            for jc in range(2):
                lo, hi = int(starts[jc]) // 128, -(-int(ends[jc]) // 128)
                tcs.append(list(range(lo, max(hi, lo + 1))))
            cores.append({"b": b, "j0": j0, "tcs": tcs})
    n0 = max(len(c["tcs"][0]) for c in cores)
    n1 = max(len(c["tcs"][1]) for c in cores)
    key = ("B", n0, n1)
    if key not in _cache:
        _cache[key] = build_kernel_b(n0, n1)
    nc = _cache[key]

    # host-side weight prep (pure init-time folding):
    #   rmsnorm scale rv folded into embT; qnorm and 1/sqrt(hd) into wq;
    #   kvnorm into wk/wv.
    embf = embed_W.astype(np.float32)
    rv = 1.0 / np.sqrt((embf.astype(np.float64) ** 2).mean(1) + RMS_EPS)
    embn = embf * rv[:, None].astype(np.float32)
    embTp = np.zeros((D, VP), np.float32)
    embTp[:, :V] = embn.T
    embTp = embTp.astype(ml_dtypes.bfloat16)
    wq_f = np.ascontiguousarray(
        (wq.astype(np.float32) * qnorm_w.astype(np.float32)[None, :]).T
        * (1.0 / np.sqrt(HD).astype(np.float32))).astype(ml_dtypes.bfloat16)
    wk_f = np.ascontiguousarray((wk.astype(np.float32)
                                 * kvnorm_w.astype(np.float32)[None, :]).T)
    wv_f = np.ascontiguousarray((wv.astype(np.float32)
                                 * kvnorm_w.astype(np.float32)[None, :]).T)
    wkv_f = np.concatenate([wk_f, wv_f], axis=1).astype(ml_dtypes.bfloat16)
    wo_f = np.ascontiguousarray(wo.astype(np.float32).T).astype(ml_dtypes.bfloat16)

    NT = n0 + n1
    in_maps = []
    for c in cores:
        b = c["b"]
        tqr = np.zeros(NT * 256, np.float32)
        pidl = np.full(NT * 128, -1.0, np.float32)  # -1 never matches a j id
        qtj = tokens[b, pos[b, c["j0"]:c["j0"] + 256]].astype(np.float32)
        slot = 0
        for jc in range(2):
            lst = c["tcs"][jc]
            want = n0 if jc == 0 else n1
            for k_ in range(want):
                if k_ < len(lst):
                    tci = lst[k_]
                    sl = slice(tci * 128, (tci + 1) * 128)
                    tqr[slot * 256:slot * 256 + 128] = tokens[b, sl]
                    # rebase pid to local j index (0..255 within this core)
                    pl = (pid[b, sl] - c["j0"]).astype(np.float32)
                    tqr[slot * 256 + 128:(slot + 1) * 256] = pl
                    pidl[slot * 128:(slot + 1) * 128] = pl
                else:
                    tqr[slot * 256 + 128:(slot + 1) * 256] = -1.0
                slot += 1
        in_maps.append({
            "embT": embTp, "wqT": wq_f, "wkvT": wkv_f, "woT": wo_f,
            "tqr": tqr[None, :], "qtj": qtj[None, :], "pidc": pidl,
        })
    res = run_bass_kernel_spmd(nc, in_maps, list(range(NCORES)),
                               trace=os.environ.get("KERNEL_TRACE") == "1")
    _cache["tB"] = res.exec_time_ns
    out = np.zeros((B, K, D), np.float32)
    for ci, c in enumerate(cores):
        out[c["b"], c["j0"]:c["j0"] + 256] = res.results[ci]["out"]
    return out


# revision 16
# speedup vs baseline: 1.1774x; 1.1697x over previous
"""Trainium2 kernel for nn_LocalEncoder (BLT-style local encoder).

Key structural insight: every per-token quantity (boundary logit z, rmsnorm
scale, q/k/v projections) depends only on the token ID (vocab=260), so all
dense math runs on the 260-row vocab tables instead of 16384 token rows.
Top-k boundary selection ties (same token id => bit-identical z in the fp32
reference) are broken by index, reproduced exactly on the host.

Pipeline:
  Kernel A (8 cores, DF split 8x384): zv partials = w2_slice @ silu(w1_slice @ embT)
  Host:     zv -> per-row boundary selection (stable by (-z, idx)) -> pos/pid/qtok
  Kernel B (8 cores = 4 seqs x 2 query-halves):
    - rmsnorm scales folded into embT on host; 1/8 and qnorm folded into wq;
      kvnorm folded into wk/wv.
    - preamble: vocab k|v table (fused 1536-wide) + vocab q table -> per-jc
      128-row patch query table qj.
    - per 128-token chunk: one-hot gathers (k|v from vocab, q from qj via the
      patch-membership mask transpose), per-token scores on GpSimd/DVE/ACT,
      block-diagonal softmax via one-hot scatter matmuls, wo proj at end.
    - software-pipelined so PE gathers of chunk i+1 overlap the score chain
      of chunk i (acc matmuls are deferred by one iteration).
"""

import os
import numpy as np
import ml_dtypes

from contextlib import ExitStack

import concourse.bass as bass
import concourse.bacc as bacc
import concourse.mybir as mybir
from concourse.tile import TileContext
from concourse.alu_op_type import AluOpType
from concourse.bass_utils import run_bass_kernel_spmd
from concourse.library_config import mlp
from concourse import masks

F32 = mybir.dt.float32
F32R = mybir.dt.float32r
BF16 = mybir.dt.bfloat16
I16 = mybir.dt.int16
AFT = mybir.ActivationFunctionType
AX = mybir.AxisListType

B, L, D, V, K, H, HD = 4, 4096, 768, 260, 512, 12, 64
DF = 4 * D
VP = 384          # vocab padded to 3 partition chunks
RMS_EPS = 1e-5
NCORES = 8
FSL = DF // NCORES  # 384 f-rows per core in kernel A

_cache = {}


# --------------------------------------------------------------------------- #
# Kernel A: per-core partial zv over a DF slice (fp32 matmuls for precision;
# the top-k threshold gap goes down to ~4e-6 so bf16/f32r would flip
# boundary selections)
# --------------------------------------------------------------------------- #
def build_kernel_a():
    nc = bacc.Bacc("TRN2", target_bir_lowering=False, debug=False)
    embT_d = nc.dram_tensor("embT", [D, V], F32, kind="ExternalInput")
    w1T_d = nc.dram_tensor("w1T", [D, FSL], F32, kind="ExternalInput")
    b1_d = nc.dram_tensor("b1s", [FSL], F32, kind="ExternalInput")
    w2_d = nc.dram_tensor("w2s", [FSL], F32, kind="ExternalInput")
    zp_d = nc.dram_tensor("zp", [1, V], F32, kind="ExternalOutput")

    with TileContext(nc) as tc:
        with (
            tc.tile_pool(name="sb", bufs=1) as sb,
            tc.tile_pool(name="ps", bufs=2, space="PSUM") as ps,
        ):
            embT = [sb.tile([128, V], F32, tag=f"embT{d}", name=f"embT{d}") for d in range(6)]
            w1T = [sb.tile([128, FSL], F32, tag=f"w1T{d}", name=f"w1T{d}") for d in range(6)]
            for d in range(6):
                nc.sync.dma_start(embT[d][:, :], embT_d[128 * d:128 * (d + 1), :])
                nc.sync.dma_start(w1T[d][:, :], w1T_d[128 * d:128 * (d + 1), :])
            b1c = sb.tile([128, 3], F32, tag="b1c")
            w2c = sb.tile([128, 3], F32, tag="w2c")
            nc.sync.dma_start(b1c[:, :], b1_d.rearrange("(i p) -> p i", p=128))
            nc.sync.dma_start(w2c[:, :], w2_d.rearrange("(i p) -> p i", p=128))

            zp_ps = ps.tile([1, V], F32, tag="zp")
            for fi in range(3):
                y1p = ps.tile([128, V], F32, tag="y1")
                for d in range(6):
                    nc.tensor.matmul(
                        y1p[:, :], w1T[d][:, 128 * fi:128 * (fi + 1)], embT[d][:, :],
                        start=(d == 0), stop=(d == 5),
                    )
                y1b = sb.tile([128, V], F32, tag="y1b")
                nc.vector.tensor_scalar(y1b[:, :], y1p[:, :], b1c[:, fi:fi + 1],
                                        None, AluOpType.add)
                sig = sb.tile([128, V], F32, tag="sig")
                nc.scalar.activation(sig[:, :], y1b[:, :], AFT.Sigmoid)
                y1s = sb.tile([128, V], F32, tag="y1s")
                nc.vector.tensor_tensor(y1s[:, :], y1b[:, :], sig[:, :],
                                        AluOpType.mult)
                nc.tensor.matmul(zp_ps[:, :], w2c[:, fi:fi + 1], y1s[:, :],
                                 start=(fi == 0), stop=(fi == 2))
            zp_s = sb.tile([1, V], F32, tag="zps")
            nc.vector.tensor_copy(zp_s[:, :], zp_ps[:, :])
            nc.sync.dma_start(zp_d[:, :], zp_s[:, :])
    nc.compile()
    return nc


def run_kernel_a(inputs):
    if "A" not in _cache:
        _cache["A"] = build_kernel_a()
    nc = _cache["A"]
    embT = np.ascontiguousarray(inputs["embed_W"].astype(np.float32).T)
    w1 = inputs["bp_w1"].astype(np.float32)
    b1 = inputs["bp_b1"].astype(np.float32)
    w2 = inputs["bp_w2"].astype(np.float32)[0]
    in_maps = []
    for c in range(NCORES):
        sl = slice(c * FSL, (c + 1) * FSL)
        in_maps.append({
            "embT": embT,
            "w1T": np.ascontiguousarray(w1[sl].T),
            "b1s": np.ascontiguousarray(b1[sl]),
            "w2s": np.ascontiguousarray(w2[sl]),
        })
    res = run_bass_kernel_spmd(nc, in_maps, list(range(NCORES)),
                               trace=os.environ.get("KERNEL_TRACE") == "1")
    _cache["tA"] = res.exec_time_ns
    zv = np.zeros(V, np.float64)
    for c in range(NCORES):
        zv += res.results[c]["zp"][0].astype(np.float64)
    zv += inputs["bp_b2"].astype(np.float64)[0]
    return zv.astype(np.float32)


# --------------------------------------------------------------------------- #
# Host boundary logic
# --------------------------------------------------------------------------- #
def boundary_plan(zv, tokens):
    """Reproduce reference top-k (stable ties by index) + patch structure."""
    zt = zv[tokens]  # [B, L]
    pos = np.zeros((B, K), np.int64)
    for b in range(B):
        key = zt[b].astype(np.float64).copy()
        key[0] = np.inf  # position 0 forced boundary (logprob set to 0 = max)
        order = np.lexsort((np.arange(L), -key))
        pos[b] = np.sort(order[:K])
    pid = (pos[:, None, :] <= np.arange(L)[None, :, None]).sum(-1) - 1  # [B, L]
    return pos, pid


# --------------------------------------------------------------------------- #
# Kernel B: sparse cross-attention via vocab tables + one-hot matmuls
# --------------------------------------------------------------------------- #
def build_kernel_b(n0, n1):
    """One NEFF for all 8 cores; n0/n1 = uniform (max-padded) chunk counts for
    the two 128-query groups. Per-core data enters via tqr/pidc/qtj/kidx.

    k|v rows are gathered per chunk by token id via SWDGE dma_gather from a
    DRAM-staged vocab table (built on PE in the preamble), freeing the PE of
    the one-hot gather matmuls."""
    NT = n0 + n1
    nc = bacc.Bacc("TRN2", target_bir_lowering=False, debug=False)

    embT_d = nc.dram_tensor("embT", [D, VP], BF16, kind="ExternalInput")
    wqT_d = nc.dram_tensor("wqT", [D, D], BF16, kind="ExternalInput")
    wkvT_d = nc.dram_tensor("wkvT", [D, 2 * D], BF16, kind="ExternalInput")
    woT_d = nc.dram_tensor("woT", [D, D], BF16, kind="ExternalInput")
    # per chunk slot: pid_local(128) as f32
    tqr_d = nc.dram_tensor("tqr", [1, NT * 128], F32R, kind="ExternalInput")
    # boundary-token vocab ids for the 256 patches (jc0 128 | jc1 128)
    qtj_d = nc.dram_tensor("qtj", [1, 256], F32R, kind="ExternalInput")
    pidc_d = nc.dram_tensor("pidc", [NT * 128], F32, kind="ExternalInput")
    # per-chunk token ids in dma_gather wrapped-idx layout
    kidx_d = nc.dram_tensor("kidx", [128, NT * 8], I16, kind="ExternalInput")
    tabkv_d = nc.dram_tensor("tabkv_s", [VP, 2 * D], BF16, kind="Internal")
    out_d = nc.dram_tensor("out", [256, D], F32, kind="ExternalOutput")

    with TileContext(nc) as tc:
        with (
            tc.tile_pool(name="sb", bufs=1) as sb,
            tc.tile_pool(name="wk", bufs=1) as wkp,
            tc.tile_pool(name="ps", bufs=1, space="PSUM") as ps,
            ExitStack() as st,
        ):
            gsem = st.enter_context(nc.semaphore("gsem"))
            tsem = st.enter_context(nc.semaphore("tsem"))
            # ---- constants ----
            ones_f = sb.tile([1, 128], F32, tag="onesf")
            nc.gpsimd.memset(ones_f[:, :], 1.0)
            ones_r = sb.tile([1, 128], F32R, tag="onesr")
            nc.vector.tensor_copy(ones_r[:, :], ones_f[:, :])
            ident = sb.tile([128, 128], F32, tag="ident")
            masks.make_identity(nc, ident[:, :])
            iotav = sb.tile([128, 3], F32, tag="iotav")   # col: p, p+128, p+256
            for vc in range(3):
                nc.gpsimd.iota(iotav[:, vc:vc + 1], [[0, 1]], base=128 * vc,
                               channel_multiplier=1,
                               allow_small_or_imprecise_dtypes=True)
            iotajc = sb.tile([128, 2], F32, tag="iotajc")  # col jc: p + 128*jc
            for jc in range(2):
                nc.gpsimd.iota(iotajc[:, jc:jc + 1], [[0, 1]], base=128 * jc,
                               channel_multiplier=1,
                               allow_small_or_imprecise_dtypes=True)
            iotaj = sb.tile([128, 256], F32, tag="iotaj")  # row 0..255 all parts
            nc.gpsimd.iota(iotaj[:, :], [[1, 256]], base=0,
                           channel_multiplier=0,
                           allow_small_or_imprecise_dtypes=True)

            # ---- load weights / tables (interleaved so build starts early) --
            embT = [sb.tile([128, VP], BF16, tag=f"embT{d}", name=f"embTb{d}") for d in range(6)]
            wq = [wkp.tile([128, D], BF16, tag=f"wq{d}", name=f"wq_{d}") for d in range(6)]
            wkv = [wkp.tile([128, 2 * D], BF16, tag=f"wkv{d}", name=f"wkv_{d}") for d in range(6)]
            wo = [wkp.tile([128, D], BF16, tag=f"wo{d}", name=f"wo_{d}") for d in range(6)]
            for d in range(6):
                nc.sync.dma_start(embT[d][:, :], embT_d[128 * d:128 * (d + 1), :])
                nc.sync.dma_start(wkv[d][:, :], wkvT_d[128 * d:128 * (d + 1), :])
            for d in range(6):
                nc.sync.dma_start(wq[d][:, :], wqT_d[128 * d:128 * (d + 1), :])
            for d in range(6):
                nc.sync.dma_start(wo[d][:, :], woT_d[128 * d:128 * (d + 1), :])
            qtj_s = sb.tile([1, 256], F32R, tag="qtj")
            nc.sync.dma_start(qtj_s[:, :], qtj_d[:, :])
            pidc = sb.tile([128, NT], F32, tag="pidc")
            nc.sync.dma_start(pidc[:, :], pidc_d.rearrange("(i p) -> p i", p=128))
            kidx = sb.tile([128, NT * 8], I16, tag="kidx")
            nc.sync.dma_start(kidx[:, :], kidx_d[:, :])

            # ---- fused vocab k|v table, staged to DRAM for dma_gather ----
            tabkv_sb = []
            for v in range(3):
                tp = ps.tile([128, 2 * D], F32, tag="tkv", name=f"tabkv_ps{v}")
                for d in range(6):
                    for s_ in range(3):
                        nc.tensor.matmul(tp[:, 512 * s_:512 * (s_ + 1)],
                                         embT[d][:, 128 * v:128 * (v + 1)],
                                         wkv[d][:, 512 * s_:512 * (s_ + 1)],
                                         start=(d == 0), stop=(d == 5))
                ts_ = sb.tile([128, 2 * D], BF16, tag="tabkv", name=f"tabkv{v}", bufs=3)
                nc.vector.tensor_copy(ts_[:, :], tp[:, :])
                tabkv_sb.append(ts_)

            # gpsimd: switch to the mlp ucode library (dma_gather), stage the
            # table to DRAM via SWDGE with an explicit completion semaphore,
            # and block gathers until it landed
            nc.gpsimd.load_library(mlp)
            for v in range(3):
                nc.gpsimd.dma_start(tabkv_d[128 * v:128 * (v + 1), :],
                                    tabkv_sb[v][:, :]).then_inc(tsem, 16)
            nc.gpsimd.wait_ge(tsem, 48)

            # ---- vocab q table (only used to extract qj below) ----
            tabq = []
            for v in range(3):
                tp = ps.tile([128, D], F32, tag="qg", name=f"tabq_ps{v}")
                for d in range(6):
                    nc.tensor.matmul(tp[:, :512],
                                     embT[d][:, 128 * v:128 * (v + 1)],
                                     wq[d][:, :512], start=(d == 0), stop=(d == 5))
                    nc.tensor.matmul(tp[:, 512:],
                                     embT[d][:, 128 * v:128 * (v + 1)],
                                     wq[d][:, 512:], start=(d == 0), stop=(d == 5))
                ts_ = sb.tile([128, D], BF16, tag=f"tabq{v}")
                nc.vector.tensor_copy(ts_[:, :], tp[:, :])
                tabq.append(ts_)

            # ---- per-jc 128-row patch query tables qj ----
            btokq = ps.tile([128, 256], F32, tag="btok", name="btokq")
            nc.tensor.matmul(btokq[:, :], ones_r[:, :], qtj_s[:, :],
                             start=True, stop=True)
            qj = []
            for jc in range(2):
                ohqj = []
                for v in range(3):
                    o = sb.tile([128, 128], BF16, tag=f"ohqj{v}_{jc}")
                    nc.vector.tensor_scalar(o[:, :],
                                            btokq[:, 128 * jc:128 * (jc + 1)],
                                            iotav[:, v:v + 1], None,
                                            AluOpType.is_equal)
                    ohqj.append(o)
                qp = ps.tile([128, D], F32, tag="qg", name=f"qj_ps{jc}")
                for v in range(3):
                    nc.tensor.matmul(qp[:, :512], ohqj[v][:, :],
                                     tabq[v][:, :512], start=(v == 0), stop=(v == 2))
                    nc.tensor.matmul(qp[:, 512:], ohqj[v][:, :],
                                     tabq[v][:, 512:], start=(v == 0), stop=(v == 2))
                qs = sb.tile([128, D], BF16, tag=f"qj{jc}")
                nc.vector.tensor_copy(qs[:, :], qp[:, :])
                qj.append(qs)

            # ---- main loop ----
            def emit_prep(tci, jc):
                """DMA + pid broadcast + mask build for chunk tci."""
                tq_s = sb.tile([1, 128], F32R, tag="tqs", name=f"tqs{tci}", bufs=3)
                nc.sync.dma_start(tq_s[:, :], tqr_d[:, 128 * tci:128 * (tci + 1)])
                bt = ps.tile([128, 128], F32, tag="btok", name=f"btok{tci}")
                nc.tensor.matmul(bt[:, :], ones_r[:, :], tq_s[:, :],
                                 start=True, stop=True)
                mtT = sb.tile([128, 128], BF16, tag="mtT", name=f"mtT{tci}", bufs=2)
                nc.vector.tensor_scalar(mtT[:, :], bt[:, :],
                                        iotajc[:, jc:jc + 1], None,
                                        AluOpType.is_equal)
                mt = sb.tile([128, 128], BF16, tag="mt", name=f"mt{tci}", bufs=3)
                nc.vector.tensor_scalar(mt[:, :],
                                        iotaj[:, 128 * jc:128 * (jc + 1)],
                                        pidc[:, tci:tci + 1], None,
                                        AluOpType.is_equal)
                return mtT, mt

            def emit_kv_gather(tci):
                """SWDGE gather of the chunk's 128 k|v rows from DRAM."""
                kvs = sb.tile([128, 2 * D], BF16, tag="kvs", name=f"kvs{tci}", bufs=3)
                nc.gpsimd.dma_gather(
                    ap3(kvs, 1, 2 * D), tabkv_d[:, :],
                    kidx[:, 8 * tci:8 * (tci + 1)], 128, 128, 2 * D,
                    elem_step=2 * D,
                ).then_inc(gsem, 16)
                return kvs

            def emit_q_gather(prep, tci):
                """PE gather of per-token q rows from the patch table."""
                mtT, _ = prep
                jc = 0 if tci < n0 else 1
                qg = ps.tile([128, D], F32, tag="qg", name=f"qg{tci}")
                nc.tensor.matmul(qg[:, :512], mtT[:, :], qj[jc][:, :512],
                                 start=True, stop=True)
                nc.tensor.matmul(qg[:, 512:], mtT[:, :], qj[jc][:, 512:],
                                 start=True, stop=True)
                return qg

            def emit_score(qg, kvs, tci):
                """score chain -> weighted values wv [128, 780] bf16."""
                qgs = sb.tile([128, D], BF16, tag="qgs", name=f"qgs{tci}", bufs=2)
                nc.scalar.copy(qgs[:, :], qg[:, :])
                # kvs data visibility: the gather's completion semaphore.
                # prod (first kvs reader) runs on vector; everything else that
                # reads kvs is ordered after prod via tile deps (gpsimd only
                # does dma_gather in the loop -- the mlp ucode library lacks
                # tensor_tensor).
                nc.vector.wait_ge(gsem, 16 * (tci + 1))
                prod = sb.tile([128, D], F32, tag="prod", name=f"prod{tci}", bufs=2)
                nc.vector.tensor_tensor(prod[:, :], qgs[:, :], kvs[:, :D],
                                        AluOpType.mult)
                s12 = sb.tile([128, H], F32, tag="s12", name=f"s12{tci}", bufs=2)
                nc.vector.tensor_reduce(
                    ap3(s12, H, 1), ap3(prod, H, HD), AX.X, AluOpType.add)
                wv = sb.tile([128, D + H], BF16, tag="wv", name=f"wv{tci}", bufs=2)
                nc.scalar.activation(wv[:, D:], s12[:, :], AFT.Exp)
                e12b = sb.tile([128, H], BF16, tag="e12b", name=f"e12b{tci}", bufs=2)
                nc.scalar.activation(e12b[:, :], s12[:, :], AFT.Exp)
                nc.vector.tensor_tensor(ap3(wv, H, HD),
                                        bcast3(e12b, H, HD),
                                        ap3(kvs, H, HD, off=D), AluOpType.mult)
                return wv

            def emit_acc(acc, mt, wv, first, last):
                nc.tensor.matmul(acc[:, :512], mt[:, :], wv[:, :512],
                                 start=first, stop=last)
                nc.tensor.matmul(acc[:, 512:], mt[:, :], wv[:, 512:],
                                 start=first, stop=last)

            def emit_finalize(acc, jc):
                zrec = sb.tile([128, H], F32, tag="zrec", name=f"zrec{jc}")
                nc.vector.reciprocal(zrec[:, :], acc[:, D:])
                pr = sb.tile([128, D], F32, tag="pr", name=f"pr{jc}")
                nc.vector.tensor_tensor(ap3(pr, H, HD), bcast3(zrec, H, HD),
                                        ap3(acc, H, HD), AluOpType.mult)
                fin = ps.tile([128, D], F32, tag="qg", name=f"fin{jc}")
                for d in range(6):
                    trp = ps.tile([128, 128], F32, tag="btok", name=f"trp{jc}_{d}")
                    nc.tensor.transpose(trp[:, :], pr[:, 128 * d:128 * (d + 1)],
                                        ident[:, :])
                    trs = sb.tile([128, 128], BF16, tag="trs", name=f"trs{jc}_{d}", bufs=2)
                    nc.vector.tensor_copy(trs[:, :], trp[:, :])
                    nc.tensor.matmul(fin[:, :512], trs[:, :], wo[d][:, :512],
                                     start=(d == 0), stop=(d == 5))
                    nc.tensor.matmul(fin[:, 512:], trs[:, :], wo[d][:, 512:],
                                     start=(d == 0), stop=(d == 5))
                fin_s = sb.tile([128, D], F32, tag="fins", name=f"fins{jc}")
                nc.vector.tensor_copy(fin_s[:, :], fin[:, :])
                nc.sync.dma_start(out_d[128 * jc:128 * (jc + 1), :], fin_s[:, :])

            # software-pipelined chunk loop: kv gathers prefetched 2 deep,
            # acc deferred one iteration so the PE never waits on the score
            # chain of the current chunk
            prep = emit_prep(0, 0)
            kvq = [emit_kv_gather(0)]
            if NT > 1:
                kvq.append(emit_kv_gather(1))
            for jc in range(2):
                ntc = n0 if jc == 0 else n1
                base = 0 if jc == 0 else n0
                acc = ps.tile([128, D + H], F32, tag="acc", name=f"acc{jc}")
                pending = None  # (mt, wv) awaiting acc
                for i in range(ntc):
                    tci = base + i
                    qg = emit_q_gather(prep, tci)
                    kvs = kvq.pop(0)
                    mt_i = prep[1]
                    if tci + 2 < NT:
                        kvq.append(emit_kv_gather(tci + 2))
                    if tci + 1 < NT:
                        prep = emit_prep(tci + 1, 0 if tci + 1 < n0 else 1)
                    if pending is not None:
                        emit_acc(acc, pending[0], pending[1], i == 1, False)
                    wv = emit_score(qg, kvs, tci)
                    pending = (mt_i, wv)
                emit_acc(acc, pending[0], pending[1], ntc == 1, True)
                emit_finalize(acc, jc)
    nc.compile()
    return nc


def ap3(tile, n, w, off=0):
    """[128, >=off+n*w] tile viewed as [128, n, w] starting at col off."""
    t = tile[:, :]
    ps_, fs = t.ap[0], t.ap[1]
    return bass.AP(t.tensor, t.offset + off * fs[0],
                   [list(ps_), [fs[0] * w, n], [fs[0], w]])


def bcast3(tile, n, w, off=0):
    """[128, n] cols (starting at off) broadcast to [128, n, w] via 0-stride."""
    t = tile[:, :]
    ps_, fs = t.ap[0], t.ap[1]
    return bass.AP(t.tensor, t.offset + off * fs[0],
                   [list(ps_), [fs[0], n], [0, w]])


# --------------------------------------------------------------------------- #
# top-level
# --------------------------------------------------------------------------- #
def kernel(tokens, embed_W, bp_w1, bp_b1, bp_w2, bp_b2, wq, wk, wv, wo,
           qnorm_w, kvnorm_w, k_patches):
    tokens = np.asarray(tokens).astype(np.int64)
    inputs = dict(tokens=tokens, embed_W=embed_W, bp_w1=bp_w1, bp_b1=bp_b1,
                  bp_w2=bp_w2, bp_b2=bp_b2)
    zv = run_kernel_a(inputs)
    pos, pid = boundary_plan(zv, tokens)

    # per-core job structure: core = 2*b + half; queries [half*256, half*256+256)
    cores = []
    for b in range(B):
        for half in range(2):
            j0 = half * 256
            ends = [pos[b, j0 + 128] if j0 + 128 < K else L,
                    pos[b, j0 + 256] if j0 + 256 < K else L]
            starts = [pos[b, j0], pos[b, j0 + 128] if j0 + 128 < K else L]
            tcs = []
            for jc in range(2):
                lo, hi = int(starts[jc]) // 128, -(-int(ends[jc]) // 128)
                tcs.append(list(range(lo, max(hi, lo + 1))))
            cores.append({"b": b, "j0": j0, "tcs": tcs})
    n0 = max(len(c["tcs"][0]) for c in cores)
    n1 = max(len(c["tcs"][1]) for c in cores)
    key = ("B", n0, n1)
    if key not in _cache:
        _cache[key] = build_kernel_b(n0, n1)
    nc = _cache[key]

    # host-side weight prep (pure init-time folding):
    #   rmsnorm scale rv folded into embT; qnorm and 1/sqrt(hd) into wq;
    #   kvnorm into wk/wv.
    embf = embed_W.astype(np.float32)
    rv = 1.0 / np.sqrt((embf.astype(np.float64) ** 2).mean(1) + RMS_EPS)
    embn = embf * rv[:, None].astype(np.float32)
    embTp = np.zeros((D, VP), np.float32)
    embTp[:, :V] = embn.T
    embTp = embTp.astype(ml_dtypes.bfloat16)
    wq_f = np.ascontiguousarray(
        (wq.astype(np.float32) * qnorm_w.astype(np.float32)[None, :]).T
        * (1.0 / np.sqrt(HD).astype(np.float32))).astype(ml_dtypes.bfloat16)
    wk_f = np.ascontiguousarray((wk.astype(np.float32)
                                 * kvnorm_w.astype(np.float32)[None, :]).T)
    wv_f = np.ascontiguousarray((wv.astype(np.float32)
                                 * kvnorm_w.astype(np.float32)[None, :]).T)
    wkv_f = np.concatenate([wk_f, wv_f], axis=1).astype(ml_dtypes.bfloat16)
    wo_f = np.ascontiguousarray(wo.astype(np.float32).T).astype(ml_dtypes.bfloat16)

    NT = n0 + n1
    wrap = (np.arange(128) % 16)[:, None], (np.arange(128) // 16)[None, :]
    in_maps = []
    for c in cores:
        b = c["b"]
        tqr = np.full(NT * 128, -1.0, np.float32)   # pid rows; -1 = no match
        pidl = np.full(NT * 128, -1.0, np.float32)
        kidx = np.zeros((128, NT * 8), np.int16)
        qtj = tokens[b, pos[b, c["j0"]:c["j0"] + 256]].astype(np.float32)
        slot = 0
        for jc in range(2):
            lst = c["tcs"][jc]
            want = n0 if jc == 0 else n1
            for k_ in range(want):
                if k_ < len(lst):
                    tci = lst[k_]
                    sl = slice(tci * 128, (tci + 1) * 128)
                    # rebase pid to local j index (0..255 within this core)
                    pl = (pid[b, sl] - c["j0"]).astype(np.float32)
                    tqr[slot * 128:(slot + 1) * 128] = pl
                    pidl[slot * 128:(slot + 1) * 128] = pl
                    # token ids in dma_gather wrapped layout: idx k at
                    # [k % 16, k // 16], replicated across 16-partition groups
                    tk = tokens[b, sl].astype(np.int16)
                    w16 = tk.reshape(8, 16).T  # [16, 8]
                    kidx[:, slot * 8:(slot + 1) * 8] = np.tile(w16, (8, 1))
                slot += 1
        in_maps.append({
            "embT": embTp, "wqT": wq_f, "wkvT": wkv_f, "woT": wo_f,
            "tqr": tqr[None, :], "qtj": qtj[None, :], "pidc": pidl,
            "kidx": kidx,
        })
    res = run_bass_kernel_spmd(nc, in_maps, list(range(NCORES)),
                               trace=os.environ.get("KERNEL_TRACE") == "1")
    _cache["tB"] = res.exec_time_ns
    out = np.zeros((B, K, D), np.float32)
    for ci, c in enumerate(cores):
        out[c["b"], c["j0"]:c["j0"] + 256] = res.results[ci]["out"]
    return out
